# revision 26
# baseline (speedup 1.0000x reference)
"""LoRA self-attention processor on 8 TRN2 NeuronCores.

Problem: B=4, S=2048, D=640, H=8 heads (hd=80), LoRA rank 4.
  q/k/v = x @ (W + up@down).T ; per-head attention; out = attn @ (Wo + o_up@o_down).T + bo

Sharding: batch*head parallel. Core c -> batch b=c//2, head-group g=c%2
(4 heads). Host folds the rank-4 LoRA updates into the weights (exact
algebra) and pre-transposes/casts operands.

Per-core pipeline (PSUM-accumulated fp32 unless noted):
  Projections: q/k as [80, S] per head (fp16, 256*sm_scale folded into
  wq on host), v as [128s, 4, 80] -> SBUF [128, 4, 81] bf16 with an
  appended ones column (softmax denominator trick).

  Attention per (chunk c of 512 q, head h): 16 score matmuls (fp16,
  k-position-major [128k, 512q]) into [128, 1024] 2-bank PSUM groups;
  softmax exp split between ACT (native Exp, exact pow-2 descale 1/256)
  and DVE (Schraudolph: i16 = z*(128/ln2)/256 + B -> uint16 convert =
  bf16 bits of exp(s), ~3% max err on that share). PV with probs as
  stationary [128,128] tiles (FWL) producing attn [128q, 81]; column 80
  is the denominator: DVE reciprocal + per-partition tensor_scalar mult
  normalizes to bf16 [q, 80]. PE transpose (identity) back to [80, q]
  for the output projection; partial out written fp32, host sums the
  two core-partials per batch + bias.

Engine balance: PE ~155us busy (the wall); exp and copies split
ACT/DVE ~100us each; GPSIMD does the tiny memsets. Software pipeline
interleaves next-iter scores with current-iter PV to keep PE fed.
"""
import numpy as np
import ml_dtypes

B, S, D, H, HD, R = 4, 2048, 640, 8, 80, 4
HPC = H // 2          # heads per core
GDIM = HPC * HD       # 320 head-dims per core
NCORES = 8
NKT = S // 128        # 16 key tiles
NQC = S // 512        # 4 query chunks
NCT = D // 128        # 5 contraction tiles
NGG = NKT // 2        # 8 exp groups of 2 k-tiles
SM_SCALE = 1.0 / float(np.sqrt(HD))
FOLD_Q = 256.0 * SM_SCALE          # folded into wq on host
EXP_SCALE = 1.0 / 256.0            # ACT descale (power of 2: exact)
SCH_MUL = float((128.0 / np.log(2.0)) / 256.0)
SCH_ADD = 16250.5                  # bf16 exp bias + minimax shift
DVE_GROUPS = frozenset()  # Schraudolph DVE path: unstable on HW, disabled

import os
KMODE = os.environ.get("KMODE", "full")  # bisect: proj | noschrau | full
if KMODE == "noschrau":
    DVE_GROUPS = frozenset()
elif KMODE.startswith("dve"):
    DVE_GROUPS = frozenset(int(ch) for ch in KMODE[3:])
SCH_U16TILE = os.environ.get("SCH_U16TILE", "0") == "1"
QKDT = os.environ.get("QKDT", "bf16")  # fp16 | bf16 for x/wqk/wv/qk_sb
TPMODE = os.environ.get("TP", "pe")  # pe | dma attnT transpose path

_cache = {}


def _body(tc, xT, w_qk, w_v, w_o, zpad, idm, outT):
    import concourse.mybir as mybir

    nc = tc.nc
    bf = mybir.dt.bfloat16
    f16 = mybir.dt.float16 if QKDT == "fp16" else mybir.dt.bfloat16
    f32 = mybir.dt.float32
    u16 = mybir.dt.uint16
    Exp = mybir.ActivationFunctionType.Exp
    Add = mybir.AluOpType.add
    Mult = mybir.AluOpType.mult

    # alternate psum->sbuf copies between ACT and DVE
    flip = [0]

    def copy_alt(out, in_):
        flip[0] ^= 1
        if flip[0]:
            nc.scalar.copy(out=out, in_=in_)
        else:
            nc.vector.tensor_copy(out=out, in_=in_)

    with tc.tile_pool(name="weights", bufs=1) as wpool, \
         tc.tile_pool(name="persist", bufs=1) as pers:
        # xT first halves + wv first: the first v-projection only needs those
        xT_t = []
        for i in range(NCT):
            t = pers.tile([128, S], f16, name=f"xT{i}", tag=f"xT{i}")
            xT_t.append(t)
        for i in range(NCT):
            nc.sync.dma_start(out=xT_t[i][:, 0:1024],
                              in_=xT[128 * i:128 * (i + 1), 0:1024])
        wv_t = []
        for i in range(NCT):
            t = wpool.tile([128, GDIM], f16, name=f"wv{i}", tag=f"wv{i}")
            nc.sync.dma_start(out=t, in_=w_v[128 * i:128 * (i + 1), :])
            wv_t.append(t)
        wqk_t = []
        for i in range(NCT):
            t = wpool.tile([128, 2 * GDIM + 48], f16, name=f"wqk{i}", tag=f"wqk{i}")
            nc.sync.dma_start(out=t, in_=w_qk[128 * i:128 * (i + 1), :])
            wqk_t.append(t)
        for i in range(NCT):
            nc.sync.dma_start(out=xT_t[i][:, 1024:2048],
                              in_=xT[128 * i:128 * (i + 1), 1024:2048])
        wo_t = []
        for i in range(3):
            t = wpool.tile([128, D], bf, name=f"wo{i}", tag=f"wo{i}")
            nc.sync.dma_start(out=t, in_=w_o[128 * i:128 * (i + 1), :])
            wo_t.append(t)
        idt = wpool.tile([128, 128], bf, name="idt", tag="idt")
        nc.sync.dma_start(out=idt, in_=idm)

        qk_sb = [pers.tile([HD, S], f16, name=f"qkT{i}", tag=f"qkT{i}")
                 for i in range(2 * HPC)]
        vcomb = [pers.tile([128, HPC, HD + 1], bf, name=f"vc{s}", tag=f"vc{s}")
                 for s in range(NKT)]
        for s in range(NKT):
            nc.gpsimd.memset(vcomb[s][:, :, HD:HD + 1], 1.0)
        attnT = [pers.tile([128, S], bf, name=f"anp{i}", tag=f"anp{i}")
                 for i in range(3)]
        # zero the four 16-row pad strips (96h+80 .. 96h+96) via DMA
        for h in range(HPC):
            t, r = divmod(96 * h + HD, 128)
            nc.sync.dma_start(out=attnT[t][r:r + 16, :], in_=zpad)

        # ---------------- v/k projections ----------------
        with tc.tile_pool(name="pjps", bufs=3, space="PSUM") as pjps:
            # v projection: [128s, 4, 80], lhsT = xT (128 cols -> FWL)
            for s in range(NKT):
                pv = pjps.tile([128, HPC, HD], f32, name="vps", tag="vps")
                for k in range(NCT):
                    nc.tensor.matmul(pv, xT_t[k][:, 128 * s:128 * (s + 1)],
                                     wv_t[k], start=(k == 0), stop=(k == NCT - 1))
                nc.vector.tensor_copy(out=vcomb[s][:, :, 0:HD], in_=pv)
            # k projections only (hh 4..7); q is projected inside the
            # attention pipeline where PE has idle gaps (ACT-bound phase)
            for hh in range(HPC, 2 * HPC):
                for c in range(NQC):
                    cs = slice(512 * c, 512 * (c + 1))
                    ps_ = pjps.tile([128, 512], f32, name="qkps", tag="qkps")
                    for k in range(NCT):
                        nc.tensor.matmul(
                            ps_, wqk_t[k][:, HD * hh:HD * hh + 128], xT_t[k][:, cs],
                            start=(k == 0), stop=(k == NCT - 1))
                    nc.vector.tensor_copy(out=qk_sb[hh][:, cs], in_=ps_[0:HD, :])

        # ---------------- attention + out projection ----------------
        if KMODE == "proj":
            return
        with tc.tile_pool(name="scps", bufs=2, space="PSUM") as scps, \
             tc.tile_pool(name="atps", bufs=2, space="PSUM") as atps, \
             tc.tile_pool(name="probs", bufs=2) as prpool, \
             tc.tile_pool(name="anp", bufs=8) as anpool, \
             tc.tile_pool(name="tstg", bufs=3) as tstg, \
             tc.tile_pool(name="rpp", bufs=8) as rppool, \
             tc.tile_pool(name="obp", bufs=3) as obpool:

            iters = [(c, h) for c in range(NQC) for h in range(HPC)]
            GRP = [3, 3, 3, 3, 2, 2]   # k-tiles per exp group (sums to 16)
            GOFF = [0, 3, 6, 9, 12, 14]
            pbs_cur, pbs_prev = [None] * NKT, [None] * NKT

            def emit_qproj(c, h):
                # q projection for (h, c), psum slot shared with out-proj
                cs = slice(512 * c, 512 * (c + 1))
                ps_ = atps.tile([128, 512], f32, name="qps", tag="at")
                for k in range(NCT):
                    nc.tensor.matmul(
                        ps_, wqk_t[k][:, HD * h:HD * h + 128], xT_t[k][:, cs],
                        start=(k == 0), stop=(k == NCT - 1))
                nc.vector.tensor_copy(out=qk_sb[h][:, cs], in_=ps_[0:HD, :])

            def emit_score_group(c, h, kk):
                cs = slice(512 * c, 512 * (c + 1))
                n = GRP[kk]
                sc = scps.tile([128, 512 * n], f32, name="sc", tag="sc",
                               padded_shape=[128, 1536])
                for p in range(n):
                    k = GOFF[kk] + p
                    nc.tensor.matmul(sc[:, 512 * p:512 * (p + 1)],
                                     qk_sb[HPC + h][:, 128 * k:128 * (k + 1)],
                                     qk_sb[h][:, cs], start=True, stop=True)
                pb = prpool.tile([128, 512 * n], bf, name="pb", tag=f"pb{kk}",
                                 padded_shape=[128, 1536])
                if kk in DVE_GROUPS:
                    nc.vector.tensor_scalar(
                        out=pb.bitcast(u16), in0=sc, scalar1=SCH_MUL,
                        scalar2=SCH_ADD, op0=Mult, op1=Add)
                else:
                    nc.scalar.activation(out=pb, in_=sc, func=Exp,
                                         scale=EXP_SCALE)
                for p in range(n):
                    pbs_cur[GOFF[kk] + p] = pb[:, 512 * p:512 * (p + 1)]

            def emit_pv_chunk(c, h, j):
                # attn [128q, 81] for q-tile j of chunk c
                ap_ = atps.tile([128, HD + 1], f32, name="ap", tag="at")
                for k in range(NKT):
                    lhsT = pbs_prev[k][:, 128 * j:128 * (j + 1)]
                    nc.tensor.matmul(ap_, lhsT, vcomb[k][:, h:h + 1, :],
                                     start=(k == 0), stop=(k == NKT - 1))
                rp = rppool.tile([128, 1], f32, name="rp", tag="rp")
                nc.vector.reciprocal(out=rp, in_=ap_[:, HD:HD + 1])
                an = anpool.tile([128, 128], bf, name="an", tag="an")
                nc.vector.tensor_scalar(out=an[:, 0:HD], in0=ap_[:, 0:HD],
                                        scalar1=rp, scalar2=None, op0=Mult)
                return an

            def emit_transposes(c, h, ans):
                cs = slice(512 * c, 512 * (c + 1))
                if TPMODE == "dma":
                    tp = tstg.tile([128, 512], bf, name="tp", tag="tstg")
                    for j in range(4):
                        nc.sync.dma_start_transpose(
                            out=tp[:, 128 * j:128 * (j + 1)], in_=ans[j])
                else:
                    tp = atps.tile([128, 512], bf, name="tp", tag="at")
                    for j in range(4):
                        nc.tensor.matmul(tp[:, 128 * j:128 * (j + 1)], ans[j],
                                         idt, is_transpose=True,
                                         start=True, stop=True)
                # scatter rows into the packed attnT at offset 96h.
                # BIR: non-zero base partition allows <= 32 partitions per AP,
                # so emit 32-row chunks (all bases stay 32-aligned).
                base = 96 * h
                for r in range(0, HD, 32):
                    n = min(32, HD - r)
                    t, off = divmod(base + r, 128)
                    nc.vector.tensor_copy(out=attnT[t][off:off + n, cs],
                                          in_=tp[r:r + n, :])

            def emit_out(c):
                cs = slice(512 * c, 512 * (c + 1))
                for d in range(NCT):
                    op = atps.tile([128, 512], f32, name="op", tag="at")
                    for i in range(3):
                        nc.tensor.matmul(op, wo_t[i][:, 128 * d:128 * (d + 1)],
                                         attnT[i][:, cs],
                                         start=(i == 0), stop=(i == 2))
                    ob = obpool.tile([128, 512], f32, name="ob", tag="ob")
                    nc.vector.tensor_copy(out=ob, in_=op)
                    nc.sync.dma_start(out=outT[128 * d:128 * (d + 1), cs], in_=ob)

            # software pipeline: q-proj of iter i+1, scores of iter i,
            # PV of iter i-1 interleaved to keep PE fed while ACT exps.
            for i in range(len(iters) + 1):
                cur = iters[i] if i < len(iters) else None
                prev = iters[i - 1] if i > 0 else None
                if i == 0:
                    emit_qproj(*iters[0])  # prime: q for the first iter
                if cur is not None:
                    for kk in range(len(GRP)):
                        emit_score_group(cur[0], cur[1], kk)
                        if kk == 1 and i + 1 < len(iters):
                            # fill the exp-wait gap before group 2 can issue
                            emit_qproj(*iters[i + 1])
                elif i + 1 < len(iters):
                    emit_qproj(*iters[i + 1])
                if prev is not None:
                    ans = [emit_pv_chunk(prev[0], prev[1], j) for j in range(4)]
                    emit_transposes(prev[0], prev[1], ans)
                    if prev[1] == HPC - 1:
                        emit_out(prev[0])
                pbs_cur, pbs_prev = [None] * NKT, pbs_cur


def build_nc(loop=1):
    import concourse.mybir as mybir
    import concourse.tile as tile
    from concourse import bacc

    bf = mybir.dt.bfloat16
    f16 = mybir.dt.float16 if QKDT == "fp16" else mybir.dt.bfloat16
    f32 = mybir.dt.float32
    nc = bacc.Bacc("TRN2", target_bir_lowering=False, debug=False,
                   num_devices=NCORES)
    xT = nc.dram_tensor("xT", [D, S], f16, kind="ExternalInput").ap()
    w_qk = nc.dram_tensor("w_qk", [D, 2 * GDIM + 48], f16, kind="ExternalInput").ap()
    w_v = nc.dram_tensor("w_v", [D, GDIM], f16, kind="ExternalInput").ap()
    w_o = nc.dram_tensor("w_o", [3 * 128, D], bf, kind="ExternalInput").ap()
    zpad = nc.dram_tensor("zpad", [16, S], bf, kind="ExternalInput").ap()
    idm = nc.dram_tensor("idm", [128, 128], bf, kind="ExternalInput").ap()
    outT = nc.dram_tensor("outT", [D, S], f32, kind="ExternalOutput").ap()
    with tile.TileContext(nc) as tc:
        if loop == 1:
            _body(tc, xT, w_qk, w_v, w_o, zpad, idm, outT)
        else:
            with tc.For_i(0, loop, 1):
                _body(tc, xT, w_qk, w_v, w_o, zpad, idm, outT)
    nc.compile()
    return nc


def make_in_maps(inputs):
    """Host-side shard + layout prep. inputs: full-size fp32 arrays."""
    f = {k: np.asarray(v, dtype=np.float64) for k, v in inputs.items()}
    w_eff = {}
    for nm in ("q", "k", "v", "o"):
        w_eff[nm] = (f[f"w{nm}"] + f[f"{nm}_up"] @ f[f"{nm}_down"])
    bfd = ml_dtypes.bfloat16
    hdt = np.float16 if QKDT == "fp16" else bfd
    x = f["hidden_states"]  # [B, S, D]
    idm = np.eye(128, dtype=bfd)
    in_maps = []
    for c in range(NCORES):
        b, g = divmod(c, 2)
        rows = slice(GDIM * g, GDIM * (g + 1))
        xT = np.ascontiguousarray(x[b].T).astype(hdt)
        wq = (w_eff["q"][rows, :] * FOLD_Q).T  # [640, 320], descaled in exp
        wk = w_eff["k"][rows, :].T
        w_qk = np.ascontiguousarray(np.concatenate(
            [wq, wk, np.zeros((D, 48))], axis=1)).astype(hdt)
        w_v = np.ascontiguousarray(w_eff["v"][rows, :].T).astype(hdt)
        wo_rows = w_eff["o"][:, rows].T  # [320, 640]
        w_o = np.zeros((384, 640), np.float64)
        for h in range(HPC):
            w_o[96 * h:96 * h + HD] = wo_rows[HD * h:HD * (h + 1)]
        w_o = np.ascontiguousarray(w_o).astype(bfd)
        zp = np.zeros((16, S), bfd)
        in_maps.append({"xT": xT, "w_qk": w_qk, "w_v": w_v, "w_o": w_o,
                        "zpad": zp, "idm": idm})
    return in_maps


def assemble_out(results, bo):
    out = np.empty((B, S, D), np.float32)
    for b in range(B):
        pt = results[2 * b]["outT"] + results[2 * b + 1]["outT"]  # [640, 2048]
        out[b] = pt.T + bo[None, :].astype(np.float32)
    return out


def kernel(**inputs):
    from concourse.bass_utils import run_bass_kernel_spmd

    if "nc" not in _cache:
        _cache["nc"] = build_nc()
    nc = _cache["nc"]
    in_maps = make_in_maps(inputs)
    res = run_bass_kernel_spmd(nc, in_maps, list(range(NCORES)))
    return assemble_out(res.results, np.asarray(inputs["bo"], np.float32))


# revision 27
# speedup vs baseline: 1.0064x; 1.0064x over previous
"""LoRA self-attention processor on 8 TRN2 NeuronCores.

Problem: B=4, S=2048, D=640, H=8 heads (hd=80), LoRA rank 4.
  q/k/v = x @ (W + up@down).T ; per-head attention; out = attn @ (Wo + o_up@o_down).T + bo

Sharding: batch*head parallel. Core c -> batch b=c//2, head-group g=c%2
(4 heads). Host folds the rank-4 LoRA updates into the weights (exact
algebra) and pre-transposes/casts operands.

Per-core pipeline (PSUM-accumulated fp32 unless noted):
  Projections: q/k as [80, S] per head (fp16, 256*sm_scale folded into
  wq on host), v as [128s, 4, 80] -> SBUF [128, 4, 81] bf16 with an
  appended ones column (softmax denominator trick).

  Attention per (chunk c of 512 q, head h): 16 score matmuls (fp16,
  k-position-major [128k, 512q]) into [128, 1024] 2-bank PSUM groups;
  softmax exp split between ACT (native Exp, exact pow-2 descale 1/256)
  and DVE (Schraudolph: i16 = z*(128/ln2)/256 + B -> uint16 convert =
  bf16 bits of exp(s), ~3% max err on that share). PV with probs as
  stationary [128,128] tiles (FWL) producing attn [128q, 81]; column 80
  is the denominator: DVE reciprocal + per-partition tensor_scalar mult
  normalizes to bf16 [q, 80]. PE transpose (identity) back to [80, q]
  for the output projection; partial out written fp32, host sums the
  two core-partials per batch + bias.

Engine balance: PE ~155us busy (the wall); exp and copies split
ACT/DVE ~100us each; GPSIMD does the tiny memsets. Software pipeline
interleaves next-iter scores with current-iter PV to keep PE fed.
"""
import numpy as np
import ml_dtypes

B, S, D, H, HD, R = 4, 2048, 640, 8, 80, 4
HPC = H // 2          # heads per core
GDIM = HPC * HD       # 320 head-dims per core
NCORES = 8
NKT = S // 128        # 16 key tiles
NQC = S // 512        # 4 query chunks
NCT = D // 128        # 5 contraction tiles
NGG = NKT // 2        # 8 exp groups of 2 k-tiles
SM_SCALE = 1.0 / float(np.sqrt(HD))
FOLD_Q = 256.0 * SM_SCALE          # folded into wq on host
EXP_SCALE = 1.0 / 256.0            # ACT descale (power of 2: exact)
SCH_MUL = float((128.0 / np.log(2.0)) / 256.0)
SCH_ADD = 16250.5                  # bf16 exp bias + minimax shift
DVE_GROUPS = frozenset()  # Schraudolph DVE path: unstable on HW, disabled

import os
KMODE = os.environ.get("KMODE", "full")  # bisect: proj | noschrau | full
if KMODE == "noschrau":
    DVE_GROUPS = frozenset()
elif KMODE.startswith("dve"):
    DVE_GROUPS = frozenset(int(ch) for ch in KMODE[3:])
SCH_U16TILE = os.environ.get("SCH_U16TILE", "0") == "1"
QKDT = os.environ.get("QKDT", "bf16")  # fp16 | bf16 for x/wqk/wv/qk_sb
TPMODE = os.environ.get("TP", "pe")  # pe | dma attnT transpose path

_cache = {}


def _body(tc, xT, w_qk, w_v, w_o, zpad, idm, outT):
    import concourse.mybir as mybir

    nc = tc.nc
    bf = mybir.dt.bfloat16
    f16 = mybir.dt.float16 if QKDT == "fp16" else mybir.dt.bfloat16
    f32 = mybir.dt.float32
    u16 = mybir.dt.uint16
    Exp = mybir.ActivationFunctionType.Exp
    Add = mybir.AluOpType.add
    Mult = mybir.AluOpType.mult

    # alternate psum->sbuf copies between ACT and DVE
    flip = [0]

    def copy_alt(out, in_):
        flip[0] ^= 1
        if flip[0]:
            nc.scalar.copy(out=out, in_=in_)
        else:
            nc.vector.tensor_copy(out=out, in_=in_)

    with tc.tile_pool(name="weights", bufs=1) as wpool, \
         tc.tile_pool(name="persist", bufs=1) as pers:
        # xT first halves + wv first: the first v-projection only needs those
        xT_t = []
        for i in range(NCT):
            t = pers.tile([128, S], f16, name=f"xT{i}", tag=f"xT{i}")
            xT_t.append(t)
        for i in range(NCT):
            nc.sync.dma_start(out=xT_t[i][:, 0:1024],
                              in_=xT[128 * i:128 * (i + 1), 0:1024])
        wv_t = []
        for i in range(NCT):
            t = wpool.tile([128, GDIM], f16, name=f"wv{i}", tag=f"wv{i}")
            nc.sync.dma_start(out=t, in_=w_v[128 * i:128 * (i + 1), :])
            wv_t.append(t)
        wqk_t = []
        for i in range(NCT):
            t = wpool.tile([128, 2 * GDIM + 48], f16, name=f"wqk{i}", tag=f"wqk{i}")
            nc.sync.dma_start(out=t, in_=w_qk[128 * i:128 * (i + 1), :])
            wqk_t.append(t)
        for i in range(NCT):
            nc.sync.dma_start(out=xT_t[i][:, 1024:2048],
                              in_=xT[128 * i:128 * (i + 1), 1024:2048])
        wo_t = []
        for i in range(3):
            t = wpool.tile([128, D], bf, name=f"wo{i}", tag=f"wo{i}")
            nc.sync.dma_start(out=t, in_=w_o[128 * i:128 * (i + 1), :])
            wo_t.append(t)
        idt = wpool.tile([128, 128], bf, name="idt", tag="idt")
        nc.sync.dma_start(out=idt, in_=idm)

        qk_sb = [pers.tile([HD, S], f16, name=f"qkT{i}", tag=f"qkT{i}")
                 for i in range(2 * HPC)]
        vcomb = [pers.tile([128, HPC, HD + 1], bf, name=f"vc{s}", tag=f"vc{s}")
                 for s in range(NKT)]
        for s in range(NKT):
            nc.gpsimd.memset(vcomb[s][:, :, HD:HD + 1], 1.0)
        attnT = [pers.tile([128, S], bf, name=f"anp{i}", tag=f"anp{i}")
                 for i in range(3)]
        # zero the four 16-row pad strips (96h+80 .. 96h+96) via DMA
        for h in range(HPC):
            t, r = divmod(96 * h + HD, 128)
            nc.sync.dma_start(out=attnT[t][r:r + 16, :], in_=zpad)

        # ---------------- v/k projections ----------------
        with tc.tile_pool(name="pjps", bufs=3, space="PSUM") as pjps:
            # v projection: [128s, 4, 80], lhsT = xT (128 cols -> FWL)
            for s in range(NKT):
                pv = pjps.tile([128, HPC, HD], f32, name="vps", tag="vps")
                for k in range(NCT):
                    nc.tensor.matmul(pv, xT_t[k][:, 128 * s:128 * (s + 1)],
                                     wv_t[k], start=(k == 0), stop=(k == NCT - 1))
                nc.vector.tensor_copy(out=vcomb[s][:, :, 0:HD], in_=pv)
            # k projections only (hh 4..7); q is projected inside the
            # attention pipeline where PE has idle gaps (ACT-bound phase)
            for hh in range(HPC, 2 * HPC):
                for c in range(NQC):
                    cs = slice(512 * c, 512 * (c + 1))
                    ps_ = pjps.tile([128, 512], f32, name="qkps", tag="qkps")
                    for k in range(NCT):
                        nc.tensor.matmul(
                            ps_, wqk_t[k][:, HD * hh:HD * hh + 128], xT_t[k][:, cs],
                            start=(k == 0), stop=(k == NCT - 1))
                    nc.vector.tensor_copy(out=qk_sb[hh][:, cs], in_=ps_[0:HD, :])

        # ---------------- attention + out projection ----------------
        if KMODE == "proj":
            return
        with tc.tile_pool(name="scps", bufs=2, space="PSUM") as scps, \
             tc.tile_pool(name="atps", bufs=2, space="PSUM") as atps, \
             tc.tile_pool(name="probs", bufs=2) as prpool, \
             tc.tile_pool(name="anp", bufs=8) as anpool, \
             tc.tile_pool(name="tstg", bufs=3) as tstg, \
             tc.tile_pool(name="rpp", bufs=8) as rppool, \
             tc.tile_pool(name="obp", bufs=3) as obpool:

            iters = [(c, h) for c in range(NQC) for h in range(HPC)]
            GRP = [3, 3, 3, 3, 2, 2]   # k-tiles per exp group (sums to 16)
            GOFF = [0, 3, 6, 9, 12, 14]
            pbs_cur, pbs_prev = [None] * NKT, [None] * NKT

            def emit_qproj(c, h):
                # q projection for (h, c), psum slot shared with out-proj
                cs = slice(512 * c, 512 * (c + 1))
                ps_ = atps.tile([128, 512], f32, name="qps", tag="at")
                for k in range(NCT):
                    nc.tensor.matmul(
                        ps_, wqk_t[k][:, HD * h:HD * h + 128], xT_t[k][:, cs],
                        start=(k == 0), stop=(k == NCT - 1))
                nc.vector.tensor_copy(out=qk_sb[h][:, cs], in_=ps_[0:HD, :])

            def emit_score_group(c, h, kk):
                cs = slice(512 * c, 512 * (c + 1))
                n = GRP[kk]
                sc = scps.tile([128, 512 * n], f32, name="sc", tag="sc",
                               padded_shape=[128, 1536])
                for p in range(n):
                    k = GOFF[kk] + p
                    nc.tensor.matmul(sc[:, 512 * p:512 * (p + 1)],
                                     qk_sb[HPC + h][:, 128 * k:128 * (k + 1)],
                                     qk_sb[h][:, cs], start=True, stop=True)
                pb = prpool.tile([128, 512 * n], bf, name="pb", tag=f"pb{kk}",
                                 padded_shape=[128, 1536])
                if kk in DVE_GROUPS:
                    nc.vector.tensor_scalar(
                        out=pb.bitcast(u16), in0=sc, scalar1=SCH_MUL,
                        scalar2=SCH_ADD, op0=Mult, op1=Add)
                else:
                    nc.scalar.activation(out=pb, in_=sc, func=Exp,
                                         scale=EXP_SCALE)
                for p in range(n):
                    pbs_cur[GOFF[kk] + p] = pb[:, 512 * p:512 * (p + 1)]

            def emit_pv_chunk(c, h, j):
                # attn [128q, 81] for q-tile j of chunk c
                ap_ = atps.tile([128, HD + 1], f32, name="ap", tag="at")
                for k in range(NKT):
                    lhsT = pbs_prev[k][:, 128 * j:128 * (j + 1)]
                    nc.tensor.matmul(ap_, lhsT, vcomb[k][:, h:h + 1, :],
                                     start=(k == 0), stop=(k == NKT - 1))
                rp = rppool.tile([128, 1], f32, name="rp", tag="rp")
                nc.vector.reciprocal(out=rp, in_=ap_[:, HD:HD + 1])
                an = anpool.tile([128, 128], bf, name="an", tag="an")
                nc.vector.tensor_scalar(out=an[:, 0:HD], in0=ap_[:, 0:HD],
                                        scalar1=rp, scalar2=None, op0=Mult)
                return an

            def emit_transposes(c, h, ans):
                cs = slice(512 * c, 512 * (c + 1))
                if TPMODE == "dma":
                    tp = tstg.tile([128, 512], bf, name="tp", tag="tstg")
                    for j in range(4):
                        nc.sync.dma_start_transpose(
                            out=tp[:, 128 * j:128 * (j + 1)], in_=ans[j])
                else:
                    tp = atps.tile([128, 512], bf, name="tp", tag="at")
                    for j in range(4):
                        nc.tensor.matmul(tp[:, 128 * j:128 * (j + 1)], ans[j],
                                         idt, is_transpose=True,
                                         start=True, stop=True)
                # scatter rows into the packed attnT at offset 96h.
                # BIR: non-zero base partition allows <= 32 partitions per AP,
                # so emit 32-row chunks (all bases stay 32-aligned).
                base = 96 * h
                for r in range(0, HD, 32):
                    n = min(32, HD - r)
                    t, off = divmod(base + r, 128)
                    nc.vector.tensor_copy(out=attnT[t][off:off + n, cs],
                                          in_=tp[r:r + n, :])

            def emit_out(c):
                cs = slice(512 * c, 512 * (c + 1))
                for d in range(NCT):
                    op = atps.tile([128, 512], f32, name="op", tag="at")
                    for i in range(3):
                        nc.tensor.matmul(op, wo_t[i][:, 128 * d:128 * (d + 1)],
                                         attnT[i][:, cs],
                                         start=(i == 0), stop=(i == 2))
                    ob = obpool.tile([128, 512], f32, name="ob", tag="ob")
                    nc.vector.tensor_copy(out=ob, in_=op)
                    nc.sync.dma_start(out=outT[128 * d:128 * (d + 1), cs], in_=ob)

            # software pipeline: q-proj of iter i+1, scores of iter i,
            # PV of iter i-1 interleaved to keep PE fed while ACT exps.
            for i in range(len(iters) + 1):
                cur = iters[i] if i < len(iters) else None
                prev = iters[i - 1] if i > 0 else None
                if i == 0:
                    emit_qproj(*iters[0])  # prime: q for the first iter
                if i + 1 < len(iters):
                    emit_qproj(*iters[i + 1])
                if cur is not None:
                    for kk in range(len(GRP)):
                        emit_score_group(cur[0], cur[1], kk)
                if prev is not None:
                    ans = [emit_pv_chunk(prev[0], prev[1], j) for j in range(4)]
                    emit_transposes(prev[0], prev[1], ans)
                    if prev[1] == HPC - 1:
                        emit_out(prev[0])
                pbs_cur, pbs_prev = [None] * NKT, pbs_cur


def build_nc(loop=1):
    import concourse.mybir as mybir
    import concourse.tile as tile
    from concourse import bacc

    bf = mybir.dt.bfloat16
    f16 = mybir.dt.float16 if QKDT == "fp16" else mybir.dt.bfloat16
    f32 = mybir.dt.float32
    nc = bacc.Bacc("TRN2", target_bir_lowering=False, debug=False,
                   num_devices=NCORES)
    xT = nc.dram_tensor("xT", [D, S], f16, kind="ExternalInput").ap()
    w_qk = nc.dram_tensor("w_qk", [D, 2 * GDIM + 48], f16, kind="ExternalInput").ap()
    w_v = nc.dram_tensor("w_v", [D, GDIM], f16, kind="ExternalInput").ap()
    w_o = nc.dram_tensor("w_o", [3 * 128, D], bf, kind="ExternalInput").ap()
    zpad = nc.dram_tensor("zpad", [16, S], bf, kind="ExternalInput").ap()
    idm = nc.dram_tensor("idm", [128, 128], bf, kind="ExternalInput").ap()
    outT = nc.dram_tensor("outT", [D, S], f32, kind="ExternalOutput").ap()
    with tile.TileContext(nc) as tc:
        if loop == 1:
            _body(tc, xT, w_qk, w_v, w_o, zpad, idm, outT)
        else:
            with tc.For_i(0, loop, 1):
                _body(tc, xT, w_qk, w_v, w_o, zpad, idm, outT)
    nc.compile()
    return nc


def make_in_maps(inputs):
    """Host-side shard + layout prep. inputs: full-size fp32 arrays."""
    f = {k: np.asarray(v, dtype=np.float64) for k, v in inputs.items()}
    w_eff = {}
    for nm in ("q", "k", "v", "o"):
        w_eff[nm] = (f[f"w{nm}"] + f[f"{nm}_up"] @ f[f"{nm}_down"])
    bfd = ml_dtypes.bfloat16
    hdt = np.float16 if QKDT == "fp16" else bfd
    x = f["hidden_states"]  # [B, S, D]
    idm = np.eye(128, dtype=bfd)
    in_maps = []
    for c in range(NCORES):
        b, g = divmod(c, 2)
        rows = slice(GDIM * g, GDIM * (g + 1))
        xT = np.ascontiguousarray(x[b].T).astype(hdt)
        wq = (w_eff["q"][rows, :] * FOLD_Q).T  # [640, 320], descaled in exp
        wk = w_eff["k"][rows, :].T
        w_qk = np.ascontiguousarray(np.concatenate(
            [wq, wk, np.zeros((D, 48))], axis=1)).astype(hdt)
        w_v = np.ascontiguousarray(w_eff["v"][rows, :].T).astype(hdt)
        wo_rows = w_eff["o"][:, rows].T  # [320, 640]
        w_o = np.zeros((384, 640), np.float64)
        for h in range(HPC):
            w_o[96 * h:96 * h + HD] = wo_rows[HD * h:HD * (h + 1)]
        w_o = np.ascontiguousarray(w_o).astype(bfd)
        zp = np.zeros((16, S), bfd)
        in_maps.append({"xT": xT, "w_qk": w_qk, "w_v": w_v, "w_o": w_o,
                        "zpad": zp, "idm": idm})
    return in_maps


def assemble_out(results, bo):
    out = np.empty((B, S, D), np.float32)
    for b in range(B):
        pt = results[2 * b]["outT"] + results[2 * b + 1]["outT"]  # [640, 2048]
        out[b] = pt.T + bo[None, :].astype(np.float32)
    return out


def kernel(**inputs):
    from concourse.bass_utils import run_bass_kernel_spmd

    if "nc" not in _cache:
        _cache["nc"] = build_nc()
    nc = _cache["nc"]
    in_maps = make_in_maps(inputs)
    res = run_bass_kernel_spmd(nc, in_maps, list(range(NCORES)))
    return assemble_out(res.results, np.asarray(inputs["bo"], np.float32))
